# revision 19
# baseline (speedup 1.0000x reference)
"""Trainium2 Bass kernel for quantized BasicBlock (DoReFa conv-bn-act x2 + residual).

Self-contained: builds an 8-core SPMD Bass kernel, shards the batch (64 -> 8x8),
runs via bass_utils.run_bass_kernel_spmd, gathers the full output.

Math (per core, batch shard of 8 images):
  W_int = 2*rint(tanh(w)*s + 7.5) - 15, s = 15/(2*max|tanh(w)|)   (odd ints, |.|<=15)
  conv1: S1 = conv3x3(x, W1_int)            == 15 * conv3x3(x, w_q1)   (float32r PE pass)
  BN1 stats of S1 over (N,H,W) all-reduced across cores
  act1  = clip(rint(S1*sc1 + bi1), 0, 15)   (ints 0..15, stored fp8e4m3)
  conv2: S2 = conv3x3(act1, W2_int)         == 225 * conv3x3(a_q, w_q2), exact int fp32
         (fp8 DoubleRow: two taps contracted per matmul)
  BN2 stats of S2 all-reduced
  out   = rint(15*clip(S2*sc2 + bi2 + x, 0, 1)) / 15

Layout: each image lives in SBUF as a 58x58 zero-padded plane (+1 guard elem on
each end, IMLEN=3366). A conv "strip" is 8 output rows x 58 cols = 464 contiguous
elements; tap (dy,dx) of strip s reads the contiguous window starting at
(8s+dy)*58+dx. Seam columns (c=0,57) compute garbage; they are zeroed in PSUM
before bn_stats (so sums stay exact; counts corrected by 29/28 in the payload)
and skipped when copying valid pixels out.
"""
import sys
from contextlib import ExitStack

import numpy as np

for _p in ("/opt/trn_rl_repo",):
    if _p not in sys.path:
        sys.path.append(_p)

import concourse.bass as bass
import concourse.bass_isa as bass_isa
import concourse.bacc as bacc
import concourse.mybir as mybir
import concourse.tile as tile
from concourse import bass_utils
from concourse.ap import AP as APClass
from concourse.masks import make_identity

# Enable walrus's LDWEIGHTS dedup (concourse hardcodes it off): back-to-back
# matmuls reusing the same stationary weights then skip the reload.
_orig_run_command = bass_utils.run_command


def _patched_run_command(argv, **kwargs):
    argv = ["--enable-ldw-opt=true" if a == "--enable-ldw-opt=false" else a
            for a in argv]
    return _orig_run_command(argv, **kwargs)


# bass_utils.run_command = _patched_run_command  # walrus visitInstLdweights crash

F32 = mybir.dt.float32
F32R = mybir.dt.float32r
F16 = mybir.dt.float16
FP8 = mybir.dt.float8e4

N_CORES = 8
B, C, H, W = 64, 128, 56, 56
BPC = B // N_CORES            # images per core
WP = H + 2                    # padded row length 58
RPT = 8                       # output rows per strip
SW = RPT * WP                 # 464 elements per strip (incl seam garbage)
TPI = H // RPT                # 7 strips per image
NTILES = BPC * TPI            # 56 strips per core
IMLEN = WP * WP + 2           # 3366: 1 guard + 58*58 + 1 guard
HW = H * W                    # 3136
HWH = HW // 2
HH = H // 2
C23 = float(2 ** 23)
EPS = 1e-5
# bn payload scale: (garbage-zero-padded count per core) / (true global count)
#   per-core zpad count = 56 strips * 464 ; true global = 64*56*56
RPAY = (NTILES * SW) / float(B * H * W)   # = 29/224

TAPS = [(dy, dx) for dy in range(3) for dx in range(3)]
DR_PAIRS = [(0, 1), (2, 3), (4, 5), (6, 7)]   # tap pairs for DoubleRow; tap 8 single

_CACHE = {}


def _tap_off(s, k):
    dy, dx = TAPS[k]
    return (RPT * s + dy) * WP + dx


def _halo_zero(nc, img, base):
    """Zero the pad halo of a 58x58 padded plane at tile offset `base`
    (base points at the guard elem; image row0 starts at base+1).
    float32r views are bitcast to f32 for the memsets (zero bits identical)."""
    if img.dtype == F32R:
        img = img.bitcast(F32)
    nc.vector.memset(img[:, base:base + 60], 0.0)
    nc.vector.memset(img[:, base + 3306:base + 3366], 0.0)
    side = img[:, base + 116:base + 116 + 55 * WP].rearrange(
        "p (a b) -> p a b", b=WP)
    nc.vector.memset(side[:, :, 0:2], 0.0)


def _psum_zero_garbage(nc, ps):
    """Zero seam cols (0,57) of each of the 8 rows in a [C,464] PSUM strip."""
    b = ps[:]
    ap = APClass(b.tensor, b.offset, [list(b.ap[0]), [WP, RPT], [WP - 1, 2]])
    nc.vector.memset(ap, 0.0)


def _quant_weights(nc, pools, w_in, name):
    """DMA + DoReFa-quantize weights; returns fp32 W_int in (O, I*9) layout."""
    wp = pools["wprep"]
    wk = wp.tile([C, C * 9], F32, name=f"{name}_wk", tag="wk")
    half = C * 9 // 2
    nc.scalar.dma_start(wk[:, 0:half], w_in[:, 0:half])
    nc.scalar.dma_start(wk[:, half:], w_in[:, half:])
    am = wp.tile([C, 1], F32, name=f"{name}_am", tag="wam")
    nc.vector.tensor_reduce(am[:], wk[:], axis=mybir.AxisListType.X,
                            op=mybir.AluOpType.max, apply_absolute_value=True)
    nc.scalar.activation(wk[:], wk[:], mybir.ActivationFunctionType.Tanh)
    amg = wp.tile([C, 1], F32, name=f"{name}_amg", tag="wamg")
    nc.gpsimd.partition_all_reduce(amg[:], am[:], channels=C,
                                   reduce_op=bass_isa.ReduceOp.max)
    s_t = wp.tile([C, 1], F32, name=f"{name}_s", tag="ws")
    nc.scalar.activation(s_t[:], amg[:], mybir.ActivationFunctionType.Tanh)
    nc.vector.reciprocal(s_t[:], s_t[:])
    nc.vector.tensor_scalar_mul(s_t[:], s_t[:], 7.5)
    # W_int = 2*rint(tanh*s + 7.5) - 15
    nc.vector.tensor_scalar(wk[:], wk[:], s_t[:], 7.5,
                            op0=mybir.AluOpType.mult, op1=mybir.AluOpType.add)
    nc.vector.tensor_scalar(wk[:], wk[:], C23, C23,
                            op0=mybir.AluOpType.add, op1=mybir.AluOpType.subtract)
    nc.vector.tensor_scalar(wk[:], wk[:], 2.0, 15.0,
                            op0=mybir.AluOpType.mult, op1=mybir.AluOpType.subtract)
    return wk


def _transpose_taps(nc, pools, wint, identity, out_dt, name):
    """Per-tap PE transpose of W_int (O,(t,I)) -> wT (I,(t,O)) in out_dt."""
    wp = pools["wconst"]
    trp = pools["psum_tr"]
    wT = wp.tile([C, 9 * C], out_dt, name=f"{name}_T")
    wr = wint.rearrange("p (i t) -> p i t", t=9)
    for t in range(9):
        ps = trp.tile([C, C], F32, name=f"{name}_ps{t}", tag="trps")
        nc.tensor.transpose(ps[:], wr[:, :, t], identity[:])
        nc.scalar.copy(wT[:, t * C:(t + 1) * C], ps[:])
    return wT


def _warmup_allreduce_eps(nc, pools):
    """Tiny AllReduce at kernel start: warms up the collective stream and
    produces the BN epsilon constant (8 * 1e-5/8) with a live consumer."""
    sp = pools["stats"]
    dp = pools["dram"]
    eps8 = sp.tile([C, 1], F32, name="eps8")
    nc.vector.memset(eps8[:], EPS / N_CORES)
    cc_in = dp.tile([C, 1], F32, name="ccw_in")
    cc_out = dp.tile([C, 1], F32, name="ccw_out")
    nc.sync.dma_start(cc_in[:], eps8[:])
    nc.gpsimd.collective_compute(
        "AllReduce", mybir.AluOpType.add,
        replica_groups=[list(range(N_CORES))],
        ins=[cc_in.opt()], outs=[cc_out.opt()],
    )
    epst = sp.tile([C, 1], F32, name="epst")
    nc.sync.dma_start(epst[:], cc_out[:])
    return epst


def _bn_allreduce(nc, pools, stats, k_scale, cc_idx, epst):
    """bn_aggr local stats (of k*S, zero-padded count) -> (mean, E[x^2]) payload
    in unscaled units -> AllReduce -> mean_u, rstd_u."""
    sp = pools["stats"]
    dp = pools["dram"]
    loc = sp.tile([C, 2], F32, name=f"bn{cc_idx}_loc")
    nc.vector.bn_aggr(loc[:], stats.rearrange("p (t k) -> p t k", k=3))
    pay = sp.tile([C, 2], F32, name=f"bn{cc_idx}_pay")
    # pay0 = mean_z * RPAY / k ; pay1 = (var_z + mean_z^2) * RPAY / k^2
    nc.vector.tensor_scalar_mul(pay[:, 0:1], loc[:, 0:1], RPAY / k_scale)
    msq = sp.tile([C, 1], F32, name=f"bn{cc_idx}_msq")
    nc.vector.scalar_tensor_tensor(msq[:], loc[:, 0:1], 1.0, loc[:, 0:1],
                                   op0=mybir.AluOpType.mult, op1=mybir.AluOpType.mult)
    nc.vector.scalar_tensor_tensor(pay[:, 1:2], loc[:, 1:2], 1.0, msq[:],
                                   op0=mybir.AluOpType.bypass, op1=mybir.AluOpType.add)
    nc.vector.tensor_scalar_mul(pay[:, 1:2], pay[:, 1:2],
                                RPAY / (k_scale * k_scale))
    cc_in = dp.tile([C, 2], F32, name=f"cc{cc_idx}_in")
    cc_out = dp.tile([C, 2], F32, name=f"cc{cc_idx}_out")
    nc.sync.dma_start(cc_in[:], pay[:])
    nc.gpsimd.collective_compute(
        "AllReduce", mybir.AluOpType.add,
        replica_groups=[list(range(N_CORES))],
        ins=[cc_in.opt()], outs=[cc_out.opt()],
    )
    gs = sp.tile([C, 2], F32, name=f"bn{cc_idx}_gs")
    nc.sync.dma_start(gs[:], cc_out[:])
    mean_g = gs[:, 0:1]
    m2 = sp.tile([C, 1], F32, name=f"bn{cc_idx}_m2")
    nc.vector.scalar_tensor_tensor(m2[:], mean_g, 1.0, mean_g,
                                   op0=mybir.AluOpType.mult, op1=mybir.AluOpType.mult)
    varg = sp.tile([C, 1], F32, name=f"bn{cc_idx}_var")
    nc.vector.scalar_tensor_tensor(varg[:], m2[:], -1.0, gs[:, 1:2],
                                   op0=mybir.AluOpType.mult, op1=mybir.AluOpType.add)
    std = sp.tile([C, 1], F32, name=f"bn{cc_idx}_std")
    nc.scalar.activation(std[:], varg[:], mybir.ActivationFunctionType.Sqrt,
                         bias=epst[:])
    rstd = sp.tile([C, 1], F32, name=f"bn{cc_idx}_rstd")
    nc.vector.reciprocal(rstd[:], std[:])
    return mean_g, rstd


def _affine_vecs(nc, pools, gamma, beta, mean_u, rstd_u, m_out, k_scale, cc_idx):
    """For y_out = m*bn(S/k): sc = m*gamma*rstd/k ; bi = m*(beta - mean_u*gamma*rstd)."""
    sp = pools["stats"]
    gr = sp.tile([C, 1], F32, name=f"gr{cc_idx}")
    nc.vector.scalar_tensor_tensor(gr[:], gamma[:], 1.0, rstd_u[:],
                                   op0=mybir.AluOpType.bypass,
                                   op1=mybir.AluOpType.mult)
    sc = sp.tile([C, 1], F32, name=f"sc{cc_idx}")
    nc.vector.tensor_scalar_mul(sc[:], gr[:], m_out / k_scale)
    negms = sp.tile([C, 1], F32, name=f"negms{cc_idx}")
    nc.vector.scalar_tensor_tensor(negms[:], mean_u, -1.0, gr[:],
                                   op0=mybir.AluOpType.mult, op1=mybir.AluOpType.mult)
    bi = sp.tile([C, 1], F32, name=f"bi{cc_idx}")
    nc.vector.scalar_tensor_tensor(bi[:], negms[:], 1.0, beta[:],
                                   op0=mybir.AluOpType.bypass,
                                   op1=mybir.AluOpType.add)
    nc.vector.tensor_scalar_mul(bi[:], bi[:], m_out)
    return sc, bi


def _conv1_image(nc, pools, xpad, w1T, stats, out_sb, img_idx):
    """One image of conv1: float32r single pass, tap-outer / strip-inner."""
    cp = pools["psum_conv"]
    pss = [cp.tile([C, SW], F32, name=f"c1_ps{s}", tag="convps")
           for s in range(TPI)]
    for s in range(TPI):
        for k in range(9):
            lhsT = w1T[:, k * C:(k + 1) * C]
            st = _tap_off(s, k)
            rhs = xpad[:, st:st + SW]
            nc.tensor.matmul(pss[s][:], lhsT, rhs,
                             start=(k == 0), stop=(k == 8))
    for s in range(TPI):
        gi = img_idx * TPI + s
        _psum_zero_garbage(nc, pss[s])
        nc.vector.bn_stats(stats[:, gi * 6:(gi + 1) * 6], pss[s][:])
        psr = pss[s].rearrange("p (r c) -> p r c", c=WP)
        dst = out_sb[:, s * 448:(s + 1) * 448].rearrange(
            "p (r c) -> p r c", c=W)
        nc.scalar.copy(dst, psr[:, :, 1:57])


def _conv2_image(nc, pools, a1, a1_base, w2T, stats, out_sb, img_idx):
    """One image of conv2: fp8 DoubleRow pairs + single tap 8."""
    cp = pools["psum_conv"]
    a1f = a1[:]
    part = list(a1f.ap[0])
    pss = [cp.tile([C, SW], F32, name=f"c2_ps{s}", tag="convps")
           for s in range(TPI)]
    lhsT8 = w2T[:, 8 * C:9 * C]
    for s in range(TPI):
        for p, (ka, kb) in enumerate(DR_PAIRS):
            lhsT = w2T[:, ka * C:(kb + 1) * C].rearrange(
                "p (two m) -> p two m", two=2)
            sta = a1_base + _tap_off(s, ka)
            stb = a1_base + _tap_off(s, kb)
            rhs = APClass(a1f.tensor, a1f.offset + sta,
                          [part, [stb - sta, 2], [1, SW]])
            nc.tensor.matmul(pss[s][:], lhsT, rhs, start=(p == 0), stop=False,
                             perf_mode=mybir.MatmulPerfMode.DoubleRow)
        st = a1_base + _tap_off(s, 8)
        nc.tensor.matmul(pss[s][:], lhsT8, a1[:, st:st + SW],
                         start=False, stop=True)
    for s in range(TPI):
        gi = img_idx * TPI + s
        _psum_zero_garbage(nc, pss[s])
        nc.vector.bn_stats(stats[:, gi * 6:(gi + 1) * 6], pss[s][:])
        psr = pss[s].rearrange("p (r c) -> p r c", c=WP)
        dst = out_sb[:, s * 448:(s + 1) * 448].rearrange(
            "p (r c) -> p r c", c=W)
        nc.scalar.copy(dst, psr[:, :, 1:57])


def _act1_image(nc, o1, a1, a1_base, sc1, bi1, on_act, split=False):
    """act1 = clip(rint(o1*sc1 + bi1), 0, 15) -> fp8 interior of a1 plane.
    on_act=True: ACT Relu-affine + ACT rint, DVE clip-high/deoffset (2A+1D).
    on_act=False: all three steps on DVE (0A+3D). Mixed per image to balance
    the conv2-phase engine load."""
    interior = a1[:, a1_base + 1:a1_base + 1 + WP * WP].rearrange(
        "p (h w) -> p h w", w=WP)[:, 1:57, 1:57]
    o1r = o1.rearrange("p (h w) -> p h w", w=W)
    chunks = ([(s * RPT, (s + 1) * RPT) for s in range(TPI)]
              if split else [(0, H)])
    for r0, r1 in chunks:
        o1c = o1[:, r0 * W:r1 * W]
        o1cr = o1r[:, r0:r1, :]
        intc = interior[:, r0:r1, :]
        if on_act:
            nc.scalar.activation(o1c, o1c,
                                 mybir.ActivationFunctionType.Relu,
                                 bias=bi1[:], scale=sc1[:])
            nc.scalar.activation(o1c, o1c,
                                 mybir.ActivationFunctionType.Copy,
                                 bias=C23)
        else:
            nc.vector.tensor_scalar(o1c, o1c, sc1[:], bi1[:],
                                    op0=mybir.AluOpType.mult,
                                    op1=mybir.AluOpType.add)
            nc.vector.tensor_scalar(o1c, o1c, 0.0, C23,
                                    op0=mybir.AluOpType.max,
                                    op1=mybir.AluOpType.add)
        nc.vector.tensor_scalar(intc, o1cr, C23 + 15.0, C23,
                                op0=mybir.AluOpType.min,
                                op1=mybir.AluOpType.subtract)


def build():
    nc = bacc.Bacc("TRN2", target_bir_lowering=False, debug=False,
                   enable_asserts=False, num_devices=N_CORES)
    x_in = nc.dram_tensor("x", [BPC, C, H, W], F32, kind="ExternalInput").ap()
    w1_in = nc.dram_tensor("w1", [C, C * 9], F32, kind="ExternalInput").ap()
    w2_in = nc.dram_tensor("w2", [C, C * 9], F32, kind="ExternalInput").ap()
    g1_in = nc.dram_tensor("gamma1", [C, 1], F32, kind="ExternalInput").ap()
    b1_in = nc.dram_tensor("beta1", [C, 1], F32, kind="ExternalInput").ap()
    g2_in = nc.dram_tensor("gamma2", [C, 1], F32, kind="ExternalInput").ap()
    b2_in = nc.dram_tensor("beta2", [C, 1], F32, kind="ExternalInput").ap()
    out_d = nc.dram_tensor("out", [BPC, C, H, W], F32, kind="ExternalOutput").ap()

    with tile.TileContext(nc) as tc, ExitStack() as ctx:
        pools = {
            "wprep": ctx.enter_context(tc.tile_pool(name="wprep", bufs=1)),
            "wconst": ctx.enter_context(tc.tile_pool(name="wconst", bufs=1)),
            "stats": ctx.enter_context(tc.tile_pool(name="stats", bufs=1)),
            "big": ctx.enter_context(tc.tile_pool(name="big", bufs=8)),
            "xin": ctx.enter_context(tc.tile_pool(name="xin", bufs=2)),
            "xpad": ctx.enter_context(tc.tile_pool(name="xpad", bufs=2)),
            "a1": ctx.enter_context(tc.tile_pool(name="a1", bufs=2)),
            "xres": ctx.enter_context(tc.tile_pool(name="xres", bufs=4)),
            "psum_conv": ctx.enter_context(
                tc.tile_pool(name="psum_conv", bufs=7, space="PSUM")),
            "psum_tr": ctx.enter_context(
                tc.tile_pool(name="psum_tr", bufs=1, space="PSUM")),
            "dram": ctx.enter_context(tc.tile_pool(name="dram", bufs=4, space="DRAM")),
        }
        consts = pools["wconst"]

        # x image 0 DMA first (longest pole at startup)
        xds = []
        xd0 = pools["xin"].tile([C, HW], F32, name="xd0", tag="xin")
        x0f = x_in[0].rearrange("c h w -> c (h w)")
        nc.sync.dma_start(xd0[:, 0:HWH], x0f[:, 0:HWH])
        nc.gpsimd.dma_start(xd0[:, HWH:HW], x0f[:, HWH:HW])
        xds.append(xd0)

        # per-channel params (small, off the main queues)
        g1 = consts.tile([C, 1], F32, name="g1"); nc.gpsimd.dma_start(g1[:], g1_in[:])
        b1 = consts.tile([C, 1], F32, name="b1"); nc.gpsimd.dma_start(b1[:], b1_in[:])
        g2 = consts.tile([C, 1], F32, name="g2"); nc.gpsimd.dma_start(g2[:], g2_in[:])
        b2 = consts.tile([C, 1], F32, name="b2"); nc.gpsimd.dma_start(b2[:], b2_in[:])

        identity = consts.tile([C, C], F32, name="identity")
        make_identity(nc, identity[:])
        negc23 = consts.tile([C, 1], F32, name="negc23")
        nc.vector.memset(negc23[:], -C23)

        epst = _warmup_allreduce_eps(nc, pools)

        # ---- weights: w1 on the critical path; w2 prepped during conv1 ----
        w1i = _quant_weights(nc, pools, w1_in, "w1")
        w1T = _transpose_taps(nc, pools, w1i, identity, F16, "w1")

        # ---- phase A: conv1 per image (fp32r single pass) ----
        stats1 = pools["stats"].tile([C, NTILES * 6], F32, name="stats1")
        out1 = []
        w2T = None
        for n in range(BPC):
            if n + 1 < BPC:
                xdn = pools["xin"].tile([C, HW], F32, name=f"xd{n+1}", tag="xin")
                nc.sync.dma_start(xdn[:], x_in[n + 1].rearrange("c h w -> c (h w)"))
                xds.append(xdn)
            xpad = pools["xpad"].tile([C, IMLEN], F16, name=f"xp{n}", tag="xpad")
            _halo_zero(nc, xpad, 0)
            interior = xpad[:, 1:1 + WP * WP].rearrange(
                "p (h w) -> p h w", w=WP)[:, 1:57, 1:57]
            xdr = xds[n].rearrange("p (h w) -> p h w", w=W)
            if n == 0:
                nc.scalar.copy(interior[:, 0:HH, :], xdr[:, 0:HH, :])
                nc.scalar.copy(interior[:, HH:H, :], xdr[:, HH:H, :])
            else:
                nc.scalar.copy(interior, xdr)
            o1 = pools["big"].tile([C, HW], F32, name=f"o1_{n}", tag="bigbuf")
            _conv1_image(nc, pools, xpad, w1T, stats1, o1, n)
            out1.append(o1)
            if n == 1:
                # w2 prep rides behind the first two images' conv work
                w2i = _quant_weights(nc, pools, w2_in, "w2")
                w2T = _transpose_taps(nc, pools, w2i, identity, FP8, "w2")

        # ---- BN1 all-reduce + act1/conv2 per image ----
        mean1, rstd1 = _bn_allreduce(nc, pools, stats1, 15.0, 1, epst)
        sc1, bi1 = _affine_vecs(nc, pools, g1, b1, mean1, rstd1, 15.0, 15.0, 1)

        stats2 = pools["stats"].tile([C, NTILES * 6], F32, name="stats2")
        out2 = []
        for n in range(BPC):
            a1 = pools["a1"].tile([C, IMLEN], FP8, name=f"a1_{n}", tag="a1")
            _halo_zero(nc, a1, 0)
            _act1_image(nc, out1[n], a1, 0, sc1, bi1, on_act=(n % 8 != 3),
                        split=(n == 0))
            o2 = pools["big"].tile([C, HW], F32, name=f"o2_{n}", tag="bigbuf")
            _conv2_image(nc, pools, a1, 0, w2T, stats2, o2, n)
            out2.append(o2)

        def load_res(n):
            xra = pools["xres"].tile([C, HWH], F32, name=f"xra{n}", tag="xres")
            xrb = pools["xres"].tile([C, HWH], F32, name=f"xrb{n}", tag="xres")
            xf = x_in[n].rearrange("c h w -> c (h w)")
            nc.gpsimd.dma_start(xra[:], xf[:, 0:HWH])
            nc.gpsimd.dma_start(xrb[:], xf[:, HWH:HW])
            return xra, xrb

        # prefetch first residuals on the gpsimd queue before it blocks on AR2
        res_tiles = [load_res(0), load_res(1)]

        # ---- BN2 all-reduce + tail ----
        mean2, rstd2 = _bn_allreduce(nc, pools, stats2, 225.0, 2, epst)
        sc2, bi2 = _affine_vecs(nc, pools, g2, b2, mean2, rstd2, 1.0, 225.0, 2)
        for n in range(BPC):
            o2 = out2[n]
            xra, xrb = res_tiles[n]
            if n + 2 < BPC:
                res_tiles.append(load_res(n + 2))
            o2r = o2.rearrange("p (h w) -> p h w", w=W)
            # Per-half chains on ACT+DVE. Variant P (3A+2D) for most halves,
            # variant Q (1A+4D) every 8th half to balance engine totals.
            for hh, xrh in ((0, xra), (1, xrb)):
                sl = o2[:, hh * HWH:(hh + 1) * HWH]
                hidx = 2 * n + hh
                if hidx % 8 != 7:    # P: affine A, +x D, rint A, cliplo A, hi D
                    nc.scalar.activation(sl, sl,
                                         mybir.ActivationFunctionType.Copy,
                                         scale=sc2[:])
                    nc.vector.scalar_tensor_tensor(sl, sl, bi2[:], xrh[:],
                                                   op0=mybir.AluOpType.add,
                                                   op1=mybir.AluOpType.add)
                    nc.scalar.activation(sl, sl,
                                         mybir.ActivationFunctionType.Copy,
                                         bias=C23, scale=15.0)
                    nc.scalar.activation(sl, sl,
                                         mybir.ActivationFunctionType.Relu,
                                         bias=negc23[:])
                    nc.vector.tensor_scalar(sl, sl, 15.0, 1.0 / 15.0,
                                            op0=mybir.AluOpType.min,
                                            op1=mybir.AluOpType.mult)
                else:                # Q: all-DVE except rint
                    nc.vector.tensor_scalar(sl, sl, sc2[:], bi2[:],
                                            op0=mybir.AluOpType.mult,
                                            op1=mybir.AluOpType.add)
                    nc.vector.tensor_tensor(sl, sl, xrh[:],
                                            op=mybir.AluOpType.add)
                    nc.scalar.activation(sl, sl,
                                         mybir.ActivationFunctionType.Copy,
                                         bias=C23, scale=15.0)
                    nc.vector.tensor_scalar(sl, sl, C23, C23 + 15.0,
                                            op0=mybir.AluOpType.max,
                                            op1=mybir.AluOpType.min)
                    nc.vector.tensor_scalar(sl, sl, C23, 1.0 / 15.0,
                                            op0=mybir.AluOpType.subtract,
                                            op1=mybir.AluOpType.mult)
                nc.sync.dma_start(out_d[n][:, hh * HH:(hh + 1) * HH, :],
                                  o2r[:, hh * HH:(hh + 1) * HH, :])

    nc.compile()
    return nc


def _get_nc():
    if "nc" not in _CACHE:
        _CACHE["nc"] = build()
    return _CACHE["nc"]


def kernel(x, w1, w2, gamma1, beta1, gamma2, beta2, _trace=False):
    nc = _get_nc()
    x = np.ascontiguousarray(np.asarray(x, dtype=np.float32))
    in_common = {
        "w1": np.ascontiguousarray(np.asarray(w1, np.float32).reshape(C, C * 9)),
        "w2": np.ascontiguousarray(np.asarray(w2, np.float32).reshape(C, C * 9)),
        "gamma1": np.asarray(gamma1, np.float32).reshape(C, 1),
        "beta1": np.asarray(beta1, np.float32).reshape(C, 1),
        "gamma2": np.asarray(gamma2, np.float32).reshape(C, 1),
        "beta2": np.asarray(beta2, np.float32).reshape(C, 1),
    }
    in_maps = [dict(in_common, x=x[c * BPC:(c + 1) * BPC]) for c in range(N_CORES)]
    res = bass_utils.run_bass_kernel_spmd(nc, in_maps, core_ids=list(range(N_CORES)),
                                          trace=_trace)
    out = np.concatenate([res.results[c]["out"] for c in range(N_CORES)], axis=0)
    if _trace:
        _CACHE["last_exec_time_ns"] = res.exec_time_ns
        _CACHE["last_results"] = res
    return out


if __name__ == "__main__":
    nc = build()
    print("built ok")
